# revision 1
# baseline (speedup 1.0000x reference)
"""GATedge kernel for Trainium2, 8 NeuronCores, batch-parallel (1 batch elem / core).

Math (per batch b), restructured from the reference:
  s_e  = dot(W_edge, attn_l)                     (scalar, host)
  wl   = W_src @ attn_l, wr = W_dst @ attn_r     (host)
  el   = h_src @ wl            (1000,)           [PE, broadcast to partitions]
  er   = h_dst @ wr            (64,)             [PE]
  a    = adj*(el+er) + s_e*ef + mask  (o, m)     [DVE/ACT, m-on-partitions x2]
  eijk = lrelu(a) == max(a, 0.2a);  ekk = lrelu(2*er)
  p    = exp(eijk); Z = sum_o p + exp(ekk)       [mask -1e30 folded on host]
  F    = p^T @ feat_src, feat_src = h_src @ W_src  [PE]
  out  = sigmoid((S1*W_edge/s_e + F + pk*feat_dst) / Z)
  with S1 = sum_o p*(s_e*ef), pk = exp(ekk), feat_dst = h_dst @ W_dst

Layout: (o=1000, m=64) edge tensors pack as (128, 500): partition p = m+64h
holds o-range [500h, 500h+500). Softmax reductions are free-axis (Z fused
into the ACT exp's accum); only the final half-fold crosses partitions.
The score chain is pipelined in two column chunks (256/244) that align with
the o-chunking, and h_srcT is sent column-grouped by (chunk, half) so the
el matmuls start as soon as their slice of the DMA lands.

Hardware rules learned the hard way (violations crash the exec unit):
 - fp32 PE transpose stationary width: keep to the proven 128/116 widths
 - never read PSUM rows a matmul didn't write
 - a PE transpose must be followed by its PSUM->SBUF copy before the next
   transpose into the same bank (strict T,C,T,C interleave)
 - tensor_tensor with both SBUF inputs needs equal base partitions
   (NCC_IBIR297); use the tensor_scalar per-partition form to fold halves
 - ACT Lrelu gives wrong results (alpha semantics) - use max(x, 0.2x)
"""

import sys

for _p in ("/opt/trn_rl_repo", "/root/.axon_site/_ro/trn_rl_repo"):
    if _p not in sys.path:
        sys.path.insert(0, _p)

import numpy as np

B, NO, NM, KS, KD, D = 8, 1000, 64, 128, 64, 128
HO = NO // 2               # o per packed half
CWS = [128, 128, 128, 116]  # o-chunk widths per half
COFF = [0, 128, 256, 384]   # o-chunk offsets within a half
CCW = [256, 244]            # score-chain column-chunk widths (j01 | j23)
CCO = [0, 256]              # score-chain column-chunk offsets
N_CORES = 8
NEG_SLOPE = 0.2
BIG = 1e30

# packed param buffer column layout
P_WL, P_WS, P_WD, P_WE, P_HD, P_WR = 0, 64, 192, 320, 448, 512
P_COLS = 513

_cache = {}


def _hs_off(h, j):
    """column of chunk (h, j) in the column-grouped h_srcT layout:
    [cc0: h0j0 h0j1 h1j0 h1j1 | cc1: h0j2 h0j3 h1j2 h1j3]"""
    if j < 2:
        return 256 * h + 128 * j
    return 512 + 244 * h + 128 * (j - 2)


def _build():
    import os
    import concourse.tile as tile
    from concourse import bacc, mybir
    from concourse.tile import add_dep_helper

    stage = int(os.environ.get("BASS_GAT_STAGE", "9"))
    f32 = mybir.dt.float32
    AF = mybir.ActivationFunctionType
    ALU = mybir.AluOpType
    AX = mybir.AxisListType

    nc = bacc.Bacc("TRN2", target_bir_lowering=False, debug=False,
                   num_devices=N_CORES)

    d_hsT = nc.dram_tensor("hsT", [KS, NO], f32, kind="ExternalInput")
    d_ea = nc.dram_tensor("ea", [128, NO], f32, kind="ExternalInput")
    d_par = nc.dram_tensor("par", [128, P_COLS], f32, kind="ExternalInput")
    d_out = nc.dram_tensor("out", [NM, D], f32, kind="ExternalOutput")

    def _emit(tc, sb, ps):
        # ---- input DMAs; exec order: par, hsT-cc0, ea, hsT-cc1 ----
        s_par = sb.tile([128, P_COLS], f32, tag="par")
        nc.sync.dma_start(s_par[:], d_par[:])
        s_hsT = sb.tile([KS, NO], f32, tag="hsT")
        nc.scalar.dma_start(s_hsT[:, 0:512], d_hsT[:, 0:512])
        s_ea = sb.tile([128, NO], f32, tag="ea")
        nc.scalar.dma_start(s_ea[:], d_ea[:])
        nc.scalar.dma_start(s_hsT[:, 512:NO], d_hsT[:, 512:NO])
        wlmat = s_par[:, P_WL:P_WL + NM]
        wsrc = s_par[:, P_WS:P_WS + D]
        wdst = s_par[0:KD, P_WD:P_WD + D]
        web = s_par[0:NM, P_WE:P_WE + D]
        hdT = s_par[0:KD, P_HD:P_HD + NM]
        wrcol = s_par[0:KD, P_WR:P_WR + 1]

        # stacked 64x64 identity blocks (block h on partitions [64h,64h+64))
        s_ident = sb.tile([128, 64], f32, tag="ident")
        nc.gpsimd.memset(s_ident[:], 0.0)
        for h in range(2):
            nc.gpsimd.affine_select(
                out=s_ident[64 * h:64 * h + 64, :],
                in_=s_ident[64 * h:64 * h + 64, :],
                compare_op=ALU.not_equal,
                fill=1.0, base=0, pattern=[[-1, 64]], channel_multiplier=1)

        # ---- PE: er then el (2 matmuls per chain chunk) ----
        ps_er = ps.tile([128, 1], f32, tag="er", padded_shape=[128, 512])
        for h in range(2):
            nc.tensor.matmul(ps_er[64 * h:64 * h + 64, :], hdT,
                             wrcol, start=True, stop=True)
        s_er = sb.tile([128, 1], f32, tag="er_sb")
        nc.scalar.copy(s_er[:], ps_er[:])

        ps_el = [ps.tile([128, CCW[cc]], f32, tag=f"el{cc}",
                         name=f"ps_el{cc}", padded_shape=[128, 512])
                 for cc in range(2)]
        for cc in range(2):
            for h in range(2):
                o0 = 512 * cc + CCW[cc] * h
                nc.tensor.matmul(ps_el[cc][64 * h:64 * h + 64, :],
                                 wlmat, s_hsT[:, o0:o0 + CCW[cc]],
                                 start=True, stop=True)

        if stage <= 1:
            s_dbg = sb.tile([NM, D], f32, tag="dbg")
            nc.vector.tensor_copy(s_dbg[:], s_ea[0:NM, 0:D])
            nc.sync.dma_start(d_out[:], s_dbg[:])
            return

        # ---- score chain, pipelined in two column chunks ----
        s_E1 = sb.tile([128, HO], f32, tag="E1")
        s_X = sb.tile([128, HO], f32, tag="X")
        s_a2 = sb.tile([128, HO], f32, tag="a2")
        s_ay = sb.tile([128, HO], f32, tag="ay")
        s_eijk = sb.tile([128, HO], f32, tag="eijk")
        s_p = sb.tile([128, HO], f32, tag="p")
        s_Zh = [sb.tile([128, 1], f32, tag=f"Zh{i}", name=f"s_Zh{i}")
                for i in range(2)]
        r_last = None
        for i in range(2):
            c = slice(CCO[i], CCO[i] + CCW[i])
            ca = slice(HO + CCO[i], HO + CCO[i] + CCW[i])
            nc.scalar.activation(s_E1[:, c], ps_el[i][:],
                                 AF.Identity, bias=s_er[:])
            r_X = nc.vector.tensor_tensor(s_X[:, c], s_ea[:, ca],
                                          s_E1[:, c], ALU.mult)
            if r_last is not None:
                # keep the DVE program in chunk order; the scheduler
                # otherwise interleaves cc1 ahead of cc0's tail and
                # stalls the cc0 exp by ~2.5us
                add_dep_helper(r_X.ins, r_last.ins,
                               reason="score chain chunk order on DVE")
            # efm carries the -1e30 mask for adj=0 slots (host-folded)
            nc.vector.tensor_tensor(s_a2[:, c], s_X[:, c], s_ea[:, c],
                                    ALU.add)
            nc.vector.tensor_scalar_mul(s_ay[:, c], s_a2[:, c], NEG_SLOPE)
            r_last = nc.vector.tensor_max(s_eijk[:, c], s_a2[:, c],
                                          s_ay[:, c])
            r_exp = nc.scalar.activation(s_p[:, c], s_eijk[:, c], AF.Exp,
                                         accum_out=s_Zh[i][:])

        def after_chain(r):
            # pin small DVE ops behind the score chain so the in-order
            # DVE program can't stall on them mid-chain
            add_dep_helper(r.ins, r_last.ins,
                           reason="defer small DVE op past score chain")
            return r

        if stage <= 3:
            s_dbg = sb.tile([NM, D], f32, tag="dbg")
            nc.vector.tensor_copy(s_dbg[:], s_p[0:NM, 0:D])
            nc.sync.dma_start(d_out[:], s_dbg[:])
            return

        # ekk = lrelu(2*er), pk = exp(ekk)
        s_er2 = sb.tile([64, 1], f32, tag="er2")
        nc.scalar.mul(s_er2[:], ps_er[0:64, :], 2.0)
        s_er2s = sb.tile([64, 1], f32, tag="er2s")
        after_chain(nc.vector.tensor_scalar_mul(s_er2s[:], s_er2[:],
                                                NEG_SLOPE))
        s_ekk = sb.tile([64, 1], f32, tag="ekk")
        after_chain(nc.vector.tensor_max(s_ekk[:], s_er2[:], s_er2s[:]))
        s_pk = sb.tile([64, 1], f32, tag="pk")
        nc.scalar.activation(s_pk[:], s_ekk[:], AF.Exp)

        # S1 = sum_o p * efm (mul on idle GpSimd, reduce on DVE)
        s_junk = sb.tile([128, HO], f32, tag="junk")
        s_S1h = [sb.tile([128, 1], f32, tag=f"S1h{i}", name=f"s_S1h{i}")
                 for i in range(2)]
        for i in range(2):
            c = slice(CCO[i], CCO[i] + CCW[i])
            nc.gpsimd.tensor_tensor(s_junk[:, c], s_p[:, c], s_ea[:, c],
                                    ALU.mult)
            after_chain(nc.vector.tensor_reduce(s_S1h[i][:], s_junk[:, c],
                                                AX.X, ALU.add))

        # fold halves/chunks: Z = sum Zh + pk ; S1 = sum S1h
        s_Zq = [sb.tile([64, 1], f32, tag=f"Zq{i}", name=f"s_Zq{i}")
                for i in range(2)]
        s_S1q = [sb.tile([64, 1], f32, tag=f"S1q{i}", name=f"s_S1q{i}")
                 for i in range(2)]
        for i in range(2):
            after_chain(nc.vector.tensor_scalar(
                s_Zq[i][:], s_Zh[i][0:64, :], s_Zh[i][64:128, 0:1],
                None, ALU.add))
            after_chain(nc.vector.tensor_scalar(
                s_S1q[i][:], s_S1h[i][0:64, :], s_S1h[i][64:128, 0:1],
                None, ALU.add))
        s_Z = sb.tile([64, 1], f32, tag="Z")
        after_chain(nc.vector.tensor_add(s_Z[:], s_Zq[0][:], s_Zq[1][:]))
        after_chain(nc.vector.tensor_add(s_Z[:], s_Z[:], s_pk[:]))
        s_Zr = sb.tile([64, 1], f32, tag="Zr")
        after_chain(nc.vector.reciprocal(s_Zr[:], s_Z[:]))
        s_negZr = sb.tile([64, 1], f32, tag="negZr")
        after_chain(nc.vector.tensor_scalar_mul(s_negZr[:], s_Zr[:], -1.0))
        s_S1 = sb.tile([64, 1], f32, tag="S1")
        after_chain(nc.vector.tensor_add(s_S1[:], s_S1q[0][:], s_S1q[1][:]))

        if stage <= 4:
            s_dbg = sb.tile([NM, D], f32, tag="dbg")
            nc.vector.tensor_scalar(s_dbg[:], s_p[0:NM, 0:D], s_Z[:],
                                    None, ALU.mult)
            nc.sync.dma_start(d_out[:], s_dbg[:])
            return

        # ---- feat_src + p^T transposes + F, in two j-phases so the
        # j01 transposes/F overlap the cc1 score chain ----
        ps_fs = ps.tile([128, 8 * D], f32, tag="fs")
        s_fs = sb.tile([128, 8 * D], f32, tag="fs_sb")
        # two PSUM banks for the p^T transposes (h0 -> A, h1 -> B, with
        # bank B tag-sharing the dead el0 bank): each bank has at most one
        # un-copied transpose outstanding (the hardware constraint), but
        # the T->copy ping-pong of the two banks overlaps
        ps_pzT = ps.tile([128, 4 * 64], f32, tag="pzT")
        ps_pzT2 = ps.tile([128, 512], f32, tag="el0", name="ps_pzT2")
        s_pzT = sb.tile([128, 8 * 64], f32, tag="pzT_sb")
        ps_F = ps.tile([NM, D], f32, tag="F", padded_shape=[NM, 512])

        def fs_pair(j):
            w = CWS[j]
            for h in range(2):
                o0 = _hs_off(h, j)
                cp = 2 * j + h
                nc.tensor.matmul(ps_fs[0:w, D * cp:D * cp + D],
                                 s_hsT[:, o0:o0 + w],
                                 wsrc, start=True, stop=True)
            nc.scalar.copy(s_fs[0:w, 2 * D * j:2 * D * j + 2 * D],
                           ps_fs[0:w, 2 * D * j:2 * D * j + 2 * D])

        def tcf(j):
            w, o0 = CWS[j], COFF[j]
            for h in range(2):
                cp = 2 * j + h
                bank = ps_pzT if h == 0 else ps_pzT2
                nc.tensor.transpose(bank[0:w, 64 * j:64 * j + 64],
                                    s_p[64 * h:64 * h + 64, o0:o0 + w],
                                    s_ident[64 * h:64 * h + 64, :])
                dst = s_pzT[0:w, 64 * cp:64 * cp + 64]
                srcp = bank[0:w, 64 * j:64 * j + 64]
                if h == 0:
                    r = nc.scalar.copy(dst, srcp)
                    if j < 2:
                        # keep exp cc1 ahead of these copies in the ACT
                        # queue; F j01 is not on the critical path
                        add_dep_helper(r.ins, r_exp.ins,
                                       reason="exp cc1 before j01 T-copies")
                else:
                    after_chain(nc.vector.tensor_copy(dst, srcp))
            for h in range(2):
                cp = 2 * j + h
                nc.tensor.matmul(ps_F[:], s_pzT[0:w, 64 * cp:64 * cp + 64],
                                 s_fs[0:w, D * cp:D * cp + D],
                                 start=(cp == 0), stop=(cp == 7))

        fs_pair(0)
        fs_pair(1)
        tcf(0)
        tcf(1)
        fs_pair(2)
        fs_pair(3)
        tcf(2)
        tcf(3)
        ps_fd = ps.tile([NM, D], f32, tag="fd", padded_shape=[NM, 512])
        nc.tensor.matmul(ps_fd[:], hdT, wdst, start=True, stop=True)

        if stage <= 5:
            s_dbg = sb.tile([NM, D], f32, tag="dbg")
            nc.vector.tensor_copy(s_dbg[:], ps_F[:])
            nc.sync.dma_start(d_out[:], s_dbg[:])
            return

        # ---- combine: sigmoid(Zr*(S1*web + pk*fd + F)), exp-based ----
        s_t1 = sb.tile([NM, D], f32, tag="t1")
        after_chain(nc.vector.tensor_scalar(s_t1[:], ps_fd[:], s_pk[:],
                                            None, ALU.mult))
        s_t2 = sb.tile([NM, D], f32, tag="t2")
        after_chain(nc.vector.tensor_scalar(s_t2[:], web, s_S1[:], None,
                                            ALU.mult))
        s_t3 = sb.tile([NM, D], f32, tag="t3")
        after_chain(nc.vector.tensor_add(s_t3[:], s_t1[:], s_t2[:]))
        s_t4 = sb.tile([NM, D], f32, tag="t4")
        after_chain(nc.vector.tensor_add(s_t4[:], s_t3[:], ps_F[:]))
        # sigmoid(x) = 1/(1+exp(-x)) - avoids the Sigmoid ACT-table
        # reload (Exp is already loaded); x = t4 * Zr
        s_emq = sb.tile([NM, D], f32, tag="emq")
        nc.scalar.activation(s_emq[:], s_t4[:], AF.Exp, scale=s_negZr[:])
        s_den = sb.tile([NM, D], f32, tag="den")
        after_chain(nc.vector.tensor_scalar_add(s_den[:], s_emq[:], 1.0))
        s_out = sb.tile([NM, D], f32, tag="out_sb")
        after_chain(nc.vector.reciprocal(s_out[:], s_den[:]))
        nc.sync.dma_start(d_out[:], s_out[:])

    with tile.TileContext(nc) as tc:
        with tc.tile_pool(name="sb", bufs=1) as sb, \
             tc.tile_pool(name="ps", bufs=1, space="PSUM") as ps:
            _emit(tc, sb, ps)

    nc.compile()
    return nc


def _get_nc():
    if "nc" not in _cache:
        _cache["nc"] = _build()
    return _cache["nc"]


def _prep_core_inputs(h_src, h_dst, edge_feat, adj, W_src, W_dst, W_edge,
                      attn_l, attn_r):
    f32, f64 = np.float32, np.float64
    wl = (W_src.astype(f64) @ attn_l.astype(f64)).astype(f32)
    wr = (W_dst.astype(f64) @ attn_r.astype(f64)).astype(f32)
    s_e = float(np.dot(W_edge.astype(f64), attn_l.astype(f64)))
    s_e_safe = s_e if abs(s_e) > 1e-20 else 1e-20

    par = np.zeros((128, P_COLS), f32)
    par[:, P_WL:P_WL + NM] = np.tile(wl[:, None], (1, NM))
    par[:, P_WS:P_WS + D] = W_src
    par[0:KD, P_WD:P_WD + D] = W_dst
    par[0:NM, P_WE:P_WE + D] = np.tile(
        (W_edge.astype(f64) / s_e_safe).astype(f32)[None, :], (NM, 1))
    par[0:KD, P_WR:P_WR + 1] = wr[:, None]

    # fold the adjacency mask into the edge term: -1e30 where adj=0 makes
    # exp() zero those slots; p is then exactly 0 so S1 = sum p*efm is
    # unaffected (0 * -1e30 = -0.0)
    adj_f = adj.astype(f32)
    ef_s = ((edge_feat.astype(f64) * s_e) +
            (adj.astype(f64) - 1.0) * BIG).astype(f32)  # (B, NO, NM)

    in_maps = []
    for b in range(B):
        parb = par.copy()
        parb[0:KD, P_HD:P_HD + NM] = h_dst[b].T
        ea = np.empty((128, NO), f32)
        ea[0:NM, 0:HO] = ef_s[b, 0:HO].T
        ea[NM:128, 0:HO] = ef_s[b, HO:NO].T
        ea[0:NM, HO:NO] = adj_f[b, 0:HO].T
        ea[NM:128, HO:NO] = adj_f[b, HO:NO].T
        hsT = h_src[b].T.astype(f32)  # (128, 1000), o-major
        # column-group by (chain chunk, half): [0:256|500:756|256:500|756:]
        hsT_r = np.concatenate(
            [hsT[:, 0:256], hsT[:, 500:756], hsT[:, 256:500],
             hsT[:, 756:1000]], axis=1)
        in_maps.append({
            "hsT": np.ascontiguousarray(hsT_r),
            "ea": ea,
            "par": parb,
        })
    return in_maps


def kernel(**inputs):
    if "ope_ma_adj_batch" in inputs and "adj" not in inputs:
        inputs = dict(inputs)
        inputs["adj"] = inputs.pop("ope_ma_adj_batch")
    args = {k: np.asarray(inputs[k]) for k in
            ("h_src", "h_dst", "edge_feat", "adj", "W_src", "W_dst", "W_edge",
             "attn_l", "attn_r")}

    from concourse.bass_utils import run_bass_kernel_spmd

    nc = _get_nc()
    in_maps = _prep_core_inputs(**args)
    res = run_bass_kernel_spmd(nc, in_maps, core_ids=list(range(N_CORES)))
    out = np.stack([res.results[b]["out"] for b in range(B)], axis=0)
    return out.astype(np.float32)



# revision 3
# speedup vs baseline: 1.3900x; 1.3900x over previous
"""GATedge kernel for Trainium2, 8 NeuronCores, batch-parallel (1 batch elem / core).

Math (per batch b), restructured from the reference:
  s_e  = dot(W_edge, attn_l)                     (scalar, host)
  wl   = W_src @ attn_l, wr = W_dst @ attn_r     (host)
  el   = h_src @ wl            (1000,)           [PE, broadcast to partitions]
  er   = h_dst @ wr            (64,)             [PE]
  a    = el + er + efm         (o, m)            [ACT bias + DVE add]
  (adj multiply dropped: masked slots carry -1e30 in efm, so their pre-mask
   value is irrelevant - exp() zeroes them regardless)
  eijk = lrelu(a) == max(a, 0.2a);  ekk = lrelu(2*er)
  p    = exp(eijk); Z = sum_o p + exp(ekk)       [mask -1e30 folded on host]
  F    = p^T @ feat_src, feat_src = h_src @ W_src  [PE]
  out  = sigmoid((S1*W_edge/s_e + F + pk*feat_dst) / Z)
  with S1 = sum_o p*(s_e*ef), pk = exp(ekk), feat_dst = h_dst @ W_dst

All large tensors are bf16 (verified 8.9e-3 rel err vs the 2e-2 gate);
PSUM accumulation is fp32.

Layout: (o=1000, m=64) edge tensors pack as (128, 500): partition p = m+64h
holds o-range [500h, 500h+500). Softmax reductions are free-axis (Z fused
into the ACT exp's accum); only the final half-fold crosses partitions.
The score chain is pipelined in two column chunks (256/244) that align with
the o-chunking, and h_srcT is sent column-grouped by (chunk, half) so the
el matmuls start as soon as their slice of the DMA lands.

Hardware rules learned the hard way (violations crash the exec unit):
 - fp32 PE transpose stationary width: keep to the proven 128/116 widths
 - never read PSUM rows a matmul didn't write
 - a PE transpose must be followed by its PSUM->SBUF copy before the next
   transpose into the same bank (strict T,C,T,C interleave)
 - tensor_tensor with both SBUF inputs needs equal base partitions
   (NCC_IBIR297); use the tensor_scalar per-partition form to fold halves
 - ACT Lrelu gives wrong results (alpha semantics) - use max(x, 0.2x)
"""

import sys

for _p in ("/opt/trn_rl_repo", "/root/.axon_site/_ro/trn_rl_repo"):
    if _p not in sys.path:
        sys.path.insert(0, _p)

import numpy as np
import ml_dtypes

BF16 = ml_dtypes.bfloat16

B, NO, NM, KS, KD, D = 8, 1000, 64, 128, 64, 128
HO = NO // 2               # o per packed half
CWS = [128, 128, 128, 116]  # o-chunk widths per half
COFF = [0, 128, 256, 384]   # o-chunk offsets within a half
CCW = [256, 244]            # score-chain column-chunk widths (j01 | j23)
CCO = [0, 256]              # score-chain column-chunk offsets
N_CORES = 8
NEG_SLOPE = 0.2
BIG = 1e30

# packed param buffer column layout
P_WL, P_WS, P_WD, P_WE, P_HD, P_WR = 0, 64, 192, 320, 448, 512
P_COLS = 513

_cache = {}


def _hs_off(h, j):
    """column of chunk (h, j) in the column-grouped h_srcT layout:
    [cc0: h0j0 h0j1 h1j0 h1j1 | cc1: h0j2 h0j3 h1j2 h1j3]"""
    if j < 2:
        return 256 * h + 128 * j
    return 512 + 244 * h + 128 * (j - 2)


def _build():
    import os
    import concourse.tile as tile
    from concourse import bacc, mybir
    from concourse.tile import add_dep_helper

    stage = int(os.environ.get("BASS_GAT_STAGE", "9"))
    f32 = mybir.dt.float32
    bf = mybir.dt.bfloat16
    AF = mybir.ActivationFunctionType
    ALU = mybir.AluOpType
    AX = mybir.AxisListType

    nc = bacc.Bacc("TRN2", target_bir_lowering=False, debug=False,
                   num_devices=N_CORES)

    d_hsT = nc.dram_tensor("hsT", [KS, NO], bf, kind="ExternalInput")
    d_ea = nc.dram_tensor("ea", [128, HO], bf, kind="ExternalInput")
    d_par = nc.dram_tensor("par", [128, P_COLS], bf, kind="ExternalInput")
    d_out = nc.dram_tensor("out", [NM, D], f32, kind="ExternalOutput")

    def _emit(tc, sb, ps):
        # ---- input DMAs; exec order: par, hsT-cc0, ea, hsT-cc1 ----
        s_par = sb.tile([128, P_COLS], bf, tag="par")
        nc.sync.dma_start(s_par[:], d_par[:])
        s_hsT = sb.tile([KS, NO], bf, tag="hsT")
        nc.scalar.dma_start(s_hsT[:, 0:512], d_hsT[:, 0:512])
        s_ea = sb.tile([128, HO], bf, tag="ea")
        nc.scalar.dma_start(s_ea[:], d_ea[:])
        nc.scalar.dma_start(s_hsT[:, 512:NO], d_hsT[:, 512:NO])
        wlmat = s_par[:, P_WL:P_WL + NM]
        wsrc = s_par[:, P_WS:P_WS + D]
        wdst = s_par[0:KD, P_WD:P_WD + D]
        web = s_par[0:NM, P_WE:P_WE + D]
        hdT = s_par[0:KD, P_HD:P_HD + NM]
        wrcol = s_par[0:KD, P_WR:P_WR + 1]

        # stacked 64x64 identity blocks (block h on partitions [64h,64h+64))
        s_ident = sb.tile([128, 64], bf, tag="ident")
        nc.gpsimd.memset(s_ident[:], 0.0)
        for h in range(2):
            nc.gpsimd.affine_select(
                out=s_ident[64 * h:64 * h + 64, :],
                in_=s_ident[64 * h:64 * h + 64, :],
                compare_op=ALU.not_equal,
                fill=1.0, base=0, pattern=[[-1, 64]], channel_multiplier=1)

        # ---- PE: er then el (2 matmuls per chain chunk) ----
        ps_er = ps.tile([128, 1], f32, tag="er", padded_shape=[128, 512])
        for h in range(2):
            nc.tensor.matmul(ps_er[64 * h:64 * h + 64, :], hdT,
                             wrcol, start=True, stop=True)
        s_er = sb.tile([128, 1], f32, tag="er_sb")
        nc.scalar.copy(s_er[:], ps_er[:])

        ps_el = [ps.tile([128, CCW[cc]], f32, tag=f"el{cc}",
                         name=f"ps_el{cc}", padded_shape=[128, 512])
                 for cc in range(2)]
        for cc in range(2):
            for h in range(2):
                o0 = 512 * cc + CCW[cc] * h
                nc.tensor.matmul(ps_el[cc][64 * h:64 * h + 64, :],
                                 wlmat, s_hsT[:, o0:o0 + CCW[cc]],
                                 start=True, stop=True)

        if stage <= 1:
            s_dbg = sb.tile([NM, D], f32, tag="dbg")
            nc.vector.tensor_copy(s_dbg[:], s_ea[0:NM, 0:D])
            nc.sync.dma_start(d_out[:], s_dbg[:])
            return

        # ---- score chain, pipelined in two column chunks ----
        s_E1 = sb.tile([128, HO], bf, tag="E1")
        s_a2 = sb.tile([128, HO], bf, tag="a2")
        s_ay = sb.tile([128, HO], bf, tag="ay")
        s_eijk = sb.tile([128, HO], bf, tag="eijk")
        s_p = sb.tile([128, HO], bf, tag="p")
        s_Zh = [sb.tile([128, 1], f32, tag=f"Zh{i}", name=f"s_Zh{i}")
                for i in range(2)]
        r_last = None
        for i in range(2):
            c = slice(CCO[i], CCO[i] + CCW[i])
            nc.scalar.activation(s_E1[:, c], ps_el[i][:],
                                 AF.Identity, bias=s_er[:])
            # efm carries the -1e30 mask for adj=0 slots (host-folded)
            r_X = nc.vector.tensor_tensor(s_a2[:, c], s_E1[:, c], s_ea[:, c],
                                          ALU.add)
            if r_last is not None:
                # keep the DVE program in chunk order; the scheduler
                # otherwise interleaves cc1 ahead of cc0's tail and
                # stalls the cc0 exp by ~2.5us
                add_dep_helper(r_X.ins, r_last.ins,
                               reason="score chain chunk order on DVE")
            nc.vector.tensor_scalar_mul(s_ay[:, c], s_a2[:, c], NEG_SLOPE)
            r_last = nc.vector.tensor_max(s_eijk[:, c], s_a2[:, c],
                                          s_ay[:, c])
            r_exp = nc.scalar.activation(s_p[:, c], s_eijk[:, c], AF.Exp,
                                         accum_out=s_Zh[i][:])

        def after_chain(r):
            # pin small DVE ops behind the score chain so the in-order
            # DVE program can't stall on them mid-chain
            add_dep_helper(r.ins, r_last.ins,
                           reason="defer small DVE op past score chain")
            return r

        if stage <= 3:
            s_dbg = sb.tile([NM, D], f32, tag="dbg")
            nc.vector.tensor_copy(s_dbg[:], s_p[0:NM, 0:D])
            nc.sync.dma_start(d_out[:], s_dbg[:])
            return

        # ekk = lrelu(2*er), pk = exp(ekk)
        s_er2 = sb.tile([64, 1], f32, tag="er2")
        nc.scalar.mul(s_er2[:], ps_er[0:64, :], 2.0)
        s_er2s = sb.tile([64, 1], f32, tag="er2s")
        after_chain(nc.vector.tensor_scalar_mul(s_er2s[:], s_er2[:],
                                                NEG_SLOPE))
        s_ekk = sb.tile([64, 1], f32, tag="ekk")
        after_chain(nc.vector.tensor_max(s_ekk[:], s_er2[:], s_er2s[:]))
        s_pk = sb.tile([64, 1], f32, tag="pk")
        nc.scalar.activation(s_pk[:], s_ekk[:], AF.Exp)

        # S1 = sum_o p * efm (mul on idle GpSimd, reduce on DVE)
        s_junk = sb.tile([128, HO], bf, tag="junk")
        s_S1h = [sb.tile([128, 1], f32, tag=f"S1h{i}", name=f"s_S1h{i}")
                 for i in range(2)]
        for i in range(2):
            c = slice(CCO[i], CCO[i] + CCW[i])
            nc.gpsimd.tensor_tensor(s_junk[:, c], s_p[:, c], s_ea[:, c],
                                    ALU.mult)
            after_chain(nc.vector.tensor_reduce(s_S1h[i][:], s_junk[:, c],
                                                AX.X, ALU.add))

        # fold halves/chunks: Z = sum Zh + pk ; S1 = sum S1h
        s_Zq = [sb.tile([64, 1], f32, tag=f"Zq{i}", name=f"s_Zq{i}")
                for i in range(2)]
        s_S1q = [sb.tile([64, 1], f32, tag=f"S1q{i}", name=f"s_S1q{i}")
                 for i in range(2)]
        for i in range(2):
            after_chain(nc.vector.tensor_scalar(
                s_Zq[i][:], s_Zh[i][0:64, :], s_Zh[i][64:128, 0:1],
                None, ALU.add))
            after_chain(nc.vector.tensor_scalar(
                s_S1q[i][:], s_S1h[i][0:64, :], s_S1h[i][64:128, 0:1],
                None, ALU.add))
        s_Z = sb.tile([64, 1], f32, tag="Z")
        after_chain(nc.vector.tensor_add(s_Z[:], s_Zq[0][:], s_Zq[1][:]))
        after_chain(nc.vector.tensor_add(s_Z[:], s_Z[:], s_pk[:]))
        s_Zr = sb.tile([64, 1], f32, tag="Zr")
        after_chain(nc.vector.reciprocal(s_Zr[:], s_Z[:]))
        s_negZr = sb.tile([64, 1], f32, tag="negZr")
        after_chain(nc.vector.tensor_scalar_mul(s_negZr[:], s_Zr[:], -1.0))
        s_S1 = sb.tile([64, 1], f32, tag="S1")
        after_chain(nc.vector.tensor_add(s_S1[:], s_S1q[0][:], s_S1q[1][:]))

        if stage <= 4:
            s_dbg = sb.tile([NM, D], f32, tag="dbg")
            nc.vector.tensor_scalar(s_dbg[:], s_p[0:NM, 0:D], s_Z[:],
                                    None, ALU.mult)
            nc.sync.dma_start(d_out[:], s_dbg[:])
            return

        # ---- feat_src + p^T transposes + F, in two j-phases so the
        # j01 transposes/F overlap the cc1 score chain ----
        ps_fs = ps.tile([128, 8 * D], f32, tag="fs")
        s_fs = sb.tile([128, 8 * D], bf, tag="fs_sb")
        # two PSUM banks for the p^T transposes (h0 -> A, h1 -> B, with
        # bank B tag-sharing the dead el0 bank): each bank has at most one
        # un-copied transpose outstanding (the hardware constraint), but
        # the T->copy ping-pong of the two banks overlaps
        ps_pzT = ps.tile([128, 4 * 64], bf, tag="pzT")
        ps_pzT2 = ps.tile([128, 512], bf, tag="el0", name="ps_pzT2")
        s_pzT = sb.tile([128, 8 * 64], bf, tag="pzT_sb")
        ps_F = ps.tile([NM, D], f32, tag="F", padded_shape=[NM, 512])

        def fs_pair(j):
            w = CWS[j]
            for h in range(2):
                o0 = _hs_off(h, j)
                cp = 2 * j + h
                nc.tensor.matmul(ps_fs[0:w, D * cp:D * cp + D],
                                 s_hsT[:, o0:o0 + w],
                                 wsrc, start=True, stop=True)
            nc.scalar.copy(s_fs[0:w, 2 * D * j:2 * D * j + 2 * D],
                           ps_fs[0:w, 2 * D * j:2 * D * j + 2 * D])

        def tcf(j):
            w, o0 = CWS[j], COFF[j]
            for h in range(2):
                cp = 2 * j + h
                bank = ps_pzT if h == 0 else ps_pzT2
                nc.tensor.transpose(bank[0:w, 64 * j:64 * j + 64],
                                    s_p[64 * h:64 * h + 64, o0:o0 + w],
                                    s_ident[64 * h:64 * h + 64, :])
                dst = s_pzT[0:w, 64 * cp:64 * cp + 64]
                srcp = bank[0:w, 64 * j:64 * j + 64]
                if h == 0:
                    r = nc.scalar.copy(dst, srcp)
                    if j < 2:
                        # keep exp cc1 ahead of these copies in the ACT
                        # queue; F j01 is not on the critical path
                        add_dep_helper(r.ins, r_exp.ins,
                                       reason="exp cc1 before j01 T-copies")
                else:
                    after_chain(nc.vector.tensor_copy(dst, srcp))
            for h in range(2):
                cp = 2 * j + h
                nc.tensor.matmul(ps_F[:], s_pzT[0:w, 64 * cp:64 * cp + 64],
                                 s_fs[0:w, D * cp:D * cp + D],
                                 start=(cp == 0), stop=(cp == 7))

        fs_pair(0)
        fs_pair(1)
        tcf(0)
        tcf(1)
        fs_pair(2)
        fs_pair(3)
        tcf(2)
        tcf(3)
        ps_fd = ps.tile([NM, D], f32, tag="fd", padded_shape=[NM, 512])
        nc.tensor.matmul(ps_fd[:], hdT, wdst, start=True, stop=True)

        if stage <= 5:
            s_dbg = sb.tile([NM, D], f32, tag="dbg")
            nc.vector.tensor_copy(s_dbg[:], ps_F[:])
            nc.sync.dma_start(d_out[:], s_dbg[:])
            return

        # ---- combine: sigmoid(Zr*(S1*web + pk*fd + F)), exp-based ----
        s_t1 = sb.tile([NM, D], f32, tag="t1")
        after_chain(nc.vector.tensor_scalar(s_t1[:], ps_fd[:], s_pk[:],
                                            None, ALU.mult))
        s_t2 = sb.tile([NM, D], f32, tag="t2")
        after_chain(nc.vector.tensor_scalar(s_t2[:], web, s_S1[:], None,
                                            ALU.mult))
        s_t3 = sb.tile([NM, D], f32, tag="t3")
        after_chain(nc.vector.tensor_add(s_t3[:], s_t1[:], s_t2[:]))
        s_t4 = sb.tile([NM, D], f32, tag="t4")
        after_chain(nc.vector.tensor_add(s_t4[:], s_t3[:], ps_F[:]))
        # sigmoid(x) = 1/(1+exp(-x)) - avoids the Sigmoid ACT-table
        # reload (Exp is already loaded); x = t4 * Zr
        s_emq = sb.tile([NM, D], f32, tag="emq")
        nc.scalar.activation(s_emq[:], s_t4[:], AF.Exp, scale=s_negZr[:])
        s_den = sb.tile([NM, D], f32, tag="den")
        after_chain(nc.vector.tensor_scalar_add(s_den[:], s_emq[:], 1.0))
        s_out = sb.tile([NM, D], f32, tag="out_sb")
        after_chain(nc.vector.reciprocal(s_out[:], s_den[:]))
        nc.sync.dma_start(d_out[:], s_out[:])

    with tile.TileContext(nc) as tc:
        with tc.tile_pool(name="sb", bufs=1) as sb, \
             tc.tile_pool(name="ps", bufs=1, space="PSUM") as ps:
            _emit(tc, sb, ps)

    nc.compile()
    return nc


def _get_nc():
    if "nc" not in _cache:
        _cache["nc"] = _build()
    return _cache["nc"]


def _prep_core_inputs(h_src, h_dst, edge_feat, adj, W_src, W_dst, W_edge,
                      attn_l, attn_r):
    f32, f64 = np.float32, np.float64
    wl = (W_src.astype(f64) @ attn_l.astype(f64)).astype(f32)
    wr = (W_dst.astype(f64) @ attn_r.astype(f64)).astype(f32)
    s_e = float(np.dot(W_edge.astype(f64), attn_l.astype(f64)))
    s_e_safe = s_e if abs(s_e) > 1e-20 else 1e-20

    par = np.zeros((128, P_COLS), BF16)
    par[:, P_WL:P_WL + NM] = np.tile(wl[:, None], (1, NM))
    par[:, P_WS:P_WS + D] = W_src
    par[0:KD, P_WD:P_WD + D] = W_dst
    par[0:NM, P_WE:P_WE + D] = np.tile(
        (W_edge.astype(f64) / s_e_safe).astype(f32)[None, :], (NM, 1))
    par[0:KD, P_WR:P_WR + 1] = wr[:, None]

    # fold the adjacency mask into the edge term: -1e30 where adj=0 makes
    # exp() zero those slots; p is then exactly 0 so S1 = sum p*efm is
    # unaffected (0 * -1e30 = -0.0)
    ef_s = ((edge_feat.astype(f64) * s_e) +
            (adj.astype(f64) - 1.0) * BIG).astype(f32)  # (B, NO, NM)

    in_maps = []
    for b in range(B):
        parb = par.copy()
        parb[0:KD, P_HD:P_HD + NM] = h_dst[b].T
        ea = np.empty((128, HO), BF16)
        ea[0:NM, :] = ef_s[b, 0:HO].T
        ea[NM:128, :] = ef_s[b, HO:NO].T
        hsT = h_src[b].T.astype(BF16)  # (128, 1000), o-major
        # column-group by (chain chunk, half): [0:256|500:756|256:500|756:]
        hsT_r = np.concatenate(
            [hsT[:, 0:256], hsT[:, 500:756], hsT[:, 256:500],
             hsT[:, 756:1000]], axis=1)
        in_maps.append({
            "hsT": np.ascontiguousarray(hsT_r),
            "ea": ea,
            "par": parb,
        })
    return in_maps


def kernel(**inputs):
    if "ope_ma_adj_batch" in inputs and "adj" not in inputs:
        inputs = dict(inputs)
        inputs["adj"] = inputs.pop("ope_ma_adj_batch")
    args = {k: np.asarray(inputs[k]) for k in
            ("h_src", "h_dst", "edge_feat", "adj", "W_src", "W_dst", "W_edge",
             "attn_l", "attn_r")}

    from concourse.bass_utils import run_bass_kernel_spmd

    nc = _get_nc()
    in_maps = _prep_core_inputs(**args)
    res = run_bass_kernel_spmd(nc, in_maps, core_ids=list(range(N_CORES)))
    out = np.stack([res.results[b]["out"] for b in range(B)], axis=0)
    return out.astype(np.float32)


# revision 10
# speedup vs baseline: 1.5814x; 1.1376x over previous
"""GATedge kernel for Trainium2, 8 NeuronCores, batch-parallel (1 batch elem / core).

v2: o-on-partitions layout - transpose-free.

Math (per batch b), restructured from the reference:
  s_e  = dot(W_edge, attn_l); wl = W_src @ attn_l; wr = W_dst @ attn_r  (host)
  score[o,m] = el[o] + er[m] + efm[o,m]   (adj multiply dropped: masked
    slots carry -1e30 in efm so their pre-mask value is irrelevant)
  p    = exp(max(score, 0.2*score))       (lrelu via max)
  pk   = exp(lrelu(2*er));  Z[m] = sum_o p + pk
  F^T  = sum_c fs_c^T @ p_c  (fs = h_src @ W_src)   [PE, o-contraction]
  S1   = g^T @ 1, g = p * efm;  Zcol = p^T @ 1      [PE ones-matmuls]
  out  = sigmoid((S1*W_edge/s_e + F + pk*feat_dst) / Z)

Layout: o padded to 1024 = 8 chunks x 128 partitions. Score tensors are
(128, 512): block c cols [64c,64c+64) hold o-chunk c (partition p = o-128c).
el enters the score PSUM via matmuls with lhsT=hsT_c, rhs=wl tiled 64 wide
(broadcast across m); er via lhsT=wr tiled 128 wide, rhs=hdT (broadcast
across partitions). Pad rows carry efm=-1e30 so p=0 there.
F^T (d-part, m-free) needs ONE transpose back to m-part for the combine.

All large tensors bf16 (rel err 1.3e-2 vs the 2e-2 gate); PSUM fp32.

Hardware rules learned the hard way (violations crash the exec unit):
 - never read PSUM rows a matmul didn't write
 - a PE transpose must be followed by its PSUM->SBUF copy before the next
   transpose into the same bank
 - tensor_tensor with both SBUF inputs needs equal base partitions
 - ACT Lrelu gives wrong results (alpha semantics) - use max(x, 0.2x)
"""

import sys

for _p in ("/opt/trn_rl_repo", "/root/.axon_site/_ro/trn_rl_repo"):
    if _p not in sys.path:
        sys.path.insert(0, _p)

import numpy as np
import ml_dtypes

BF16 = ml_dtypes.bfloat16

B, NO, NM, KS, KD, D = 8, 1000, 64, 128, 64, 128
NOP = 1024                # o padded
NC = 8                    # o-chunks
SW = NC * NM              # score width = 512
N_CORES = 8
NEG_SLOPE = 0.2
BIG = 1e30

# packed param buffer column layout
P_WL, P_WS, P_WR, P_HD, P_WC, P_WD, P_WE = 0, 64, 192, 320, 384, 385, 513
P_COLS = 641

_cache = {}


def _build():
    import os
    import concourse.tile as tile
    from concourse import bacc, mybir
    from concourse.tile import add_dep_helper

    stage = int(os.environ.get("BASS_GAT_STAGE", "9"))
    f32 = mybir.dt.float32
    bf = mybir.dt.bfloat16
    AF = mybir.ActivationFunctionType
    ALU = mybir.AluOpType

    nc = bacc.Bacc("TRN2", target_bir_lowering=False, debug=False,
                   num_devices=N_CORES)

    d_hsT = nc.dram_tensor("hsT", [KS, NOP], bf, kind="ExternalInput")
    d_ea = nc.dram_tensor("ea", [128, SW], bf, kind="ExternalInput")
    d_par = nc.dram_tensor("par", [128, P_COLS], bf, kind="ExternalInput")
    d_out = nc.dram_tensor("out", [NM, D], f32, kind="ExternalOutput")

    def _emit(tc, sb, ps):
        # ---- input DMAs: par(SP), hsT0(Pool swdge), ea0(ACT), hsT1(DVE),
        # ea1(Pool swdge) ----
        s_par = sb.tile([128, P_COLS], bf, tag="par")
        nc.sync.dma_start(s_par[:], d_par[:])
        s_hsT = sb.tile([KS, NOP], bf, tag="hsT")
        nc.gpsimd.dma_start(s_hsT[:, 0:512], d_hsT[:, 0:512])
        s_ea = sb.tile([128, SW], bf, tag="ea")
        nc.scalar.dma_start(s_ea[:, 0:256], d_ea[:, 0:256])
        nc.sync.dma_start(s_hsT[:, 512:NOP], d_hsT[:, 512:NOP])
        nc.gpsimd.dma_start(s_ea[:, 256:SW], d_ea[:, 256:SW])

        wl64 = s_par[:, P_WL:P_WL + NM]
        wsrc = s_par[:, P_WS:P_WS + D]
        wr128 = s_par[0:KD, P_WR:P_WR + D]
        hdT = s_par[0:KD, P_HD:P_HD + NM]
        wrcol = s_par[0:KD, P_WC:P_WC + 1]
        wdst = s_par[0:KD, P_WD:P_WD + D]
        web = s_par[0:NM, P_WE:P_WE + D]

        s_ones = sb.tile([128, 1], bf, tag="ones")
        nc.gpsimd.memset(s_ones[:], 1.0)
        # 128x128 identity for the final F^T -> F transpose
        s_ident = sb.tile([128, 128], bf, tag="ident")
        nc.gpsimd.memset(s_ident[:], 0.0)
        nc.gpsimd.affine_select(
            out=s_ident[:], in_=s_ident[:],
            compare_op=ALU.not_equal,
            fill=1.0, base=0, pattern=[[-1, 128]], channel_multiplier=1)

        # ---- PE: er column for ekk; per chunk er+el accumulate pair
        # (er start=True must be immediately followed by its el start=False:
        # an intervening start=True on the same PSUM bank resets the group)
        ps_small = ps.tile([KD, 3], f32, tag="small", padded_shape=[KD, 512])
        nc.tensor.matmul(ps_small[:, 0:1], hdT, wrcol, start=True, stop=True)

        ps_score = ps.tile([128, SW], f32, tag="score")
        ps_fs = ps.tile([128, NC * D], f32, tag="fs")
        for c in range(NC):
            h_c = s_hsT[:, D * c:D * c + D]
            nc.tensor.matmul(ps_score[:, NM * c:NM * c + NM], wr128, hdT,
                             start=True, stop=False)
            nc.tensor.matmul(ps_score[:, NM * c:NM * c + NM], h_c, wl64,
                             start=False, stop=True)
            nc.tensor.matmul(ps_fs[:, D * c:D * c + D], h_c, wsrc,
                             start=True, stop=True)
        ps_fd = ps.tile([NM, D], f32, tag="fd", padded_shape=[NM, 512])
        nc.tensor.matmul(ps_fd[:], hdT, wdst, start=True, stop=True)

        if stage <= 1:
            s_dbg = sb.tile([NM, D], f32, tag="dbg")
            nc.vector.tensor_copy(s_dbg[:], ps_score[0:NM, 0:D])
            nc.sync.dma_start(d_out[:], s_dbg[:])
            return

        # ---- score chain, 2 column chunks of 256 ----
        s_a2 = sb.tile([128, SW], bf, tag="a2")
        s_ay = sb.tile([128, SW], bf, tag="ay")
        s_eijk = sb.tile([128, SW], bf, tag="eijk")
        s_p = sb.tile([128, SW], bf, tag="p")
        r_exp = [None, None]
        r_last = None
        for i in range(2):
            c = slice(256 * i, 256 * i + 256)
            r_a = nc.vector.tensor_tensor(s_a2[:, c], ps_score[:, c],
                                          s_ea[:, c], ALU.add)
            if r_last is not None:
                add_dep_helper(r_a.ins, r_last.ins,
                               reason="score chain chunk order on DVE")
            nc.vector.tensor_scalar_mul(s_ay[:, c], s_a2[:, c], NEG_SLOPE)
            r_last = nc.vector.tensor_max(s_eijk[:, c], s_a2[:, c],
                                          s_ay[:, c])
            r_exp[i] = nc.scalar.activation(s_p[:, c], s_eijk[:, c], AF.Exp)

        def after_chain(r):
            add_dep_helper(r.ins, r_last.ins,
                           reason="defer small DVE op past score chain")
            return r

        if stage <= 3:
            s_dbg = sb.tile([NM, D], f32, tag="dbg")
            nc.vector.tensor_copy(s_dbg[:], s_p[0:NM, 0:D])
            nc.sync.dma_start(d_out[:], s_dbg[:])
            return

        # ekk = lrelu(2*er), pk = exp(ekk) - off critical path (er only)
        s_er2 = sb.tile([64, 1], f32, tag="er2")
        nc.scalar.mul(s_er2[:], ps_small[:, 0:1], 2.0)
        s_er2s = sb.tile([64, 1], f32, tag="er2s")
        nc.vector.tensor_scalar_mul(s_er2s[:], s_er2[:], NEG_SLOPE)
        s_ekk = sb.tile([64, 1], f32, tag="ekk")
        nc.vector.tensor_max(s_ekk[:], s_er2[:], s_er2s[:])
        s_pk = sb.tile([64, 1], f32, tag="pk")
        nc.scalar.activation(s_pk[:], s_ekk[:], AF.Exp)

        # fs psum -> sbuf (bf16): ACT first half, DVE second half
        s_fs = sb.tile([128, NC * D], bf, tag="fs_sb")
        nc.scalar.copy(s_fs[:, 0:512], ps_fs[:, 0:512])
        nc.vector.tensor_copy(s_fs[:, 512:1024], ps_fs[:, 512:1024])

        # g = p * efm (for S1), per chunk on Pool
        s_g = sb.tile([128, SW], bf, tag="g")
        for i in range(2):
            c = slice(256 * i, 256 * i + 256)
            nc.gpsimd.tensor_tensor(s_g[:, c], s_p[:, c], s_ea[:, c],
                                    ALU.mult)

        if stage == 31:
            s_dbg = sb.tile([NM, D], f32, tag="dbg")
            nc.vector.tensor_copy(s_dbg[:], s_g[0:NM, 0:D])
            nc.sync.dma_start(d_out[:], s_dbg[:])
            return

        # ---- PE: F^T, Z, S1 accumulating matmuls per chunk ----
        ps_FT = ps.tile([D, NM], f32, tag="FT", padded_shape=[D, 512])
        for c in range(NC):
            nc.tensor.matmul(ps_FT[:], s_fs[:, D * c:D * c + D],
                             s_p[:, NM * c:NM * c + NM],
                             start=(c == 0), stop=(c == NC - 1))
        if stage == 32:
            s_dbg = sb.tile([NM, D], f32, tag="dbg")
            nc.gpsimd.memset(s_dbg[:], 0.0)
            nc.vector.tensor_copy(s_dbg[:, 0:NM], ps_FT[0:NM, 0:NM])
            nc.sync.dma_start(d_out[:], s_dbg[:])
            return
        for c in range(NC):
            nc.tensor.matmul(ps_small[:, 1:2], s_p[:, NM * c:NM * c + NM],
                             s_ones[:], start=(c == 0), stop=(c == NC - 1))
        for c in range(NC):
            nc.tensor.matmul(ps_small[:, 2:3], s_g[:, NM * c:NM * c + NM],
                             s_ones[:], start=(c == 0), stop=(c == NC - 1))
        if stage == 33:
            s_dbg = sb.tile([NM, D], f32, tag="dbg")
            nc.gpsimd.memset(s_dbg[:], 0.0)
            nc.vector.tensor_copy(s_dbg[:, 0:3], ps_small[:, 0:3])
            nc.sync.dma_start(d_out[:], s_dbg[:])
            return

        # F^T -> F (one transpose via PE)
        s_FT = sb.tile([D, NM], bf, tag="FT_sb")
        nc.vector.tensor_copy(s_FT[:], ps_FT[:])
        ps_F = ps.tile([NM, D], bf, tag="F", padded_shape=[NM, 512])
        nc.tensor.transpose(ps_F[:], s_FT[:], s_ident[:])

        # Z = Zcol + pk; Zr = 1/Z; negZr; S1 copy
        s_Z = sb.tile([64, 1], f32, tag="Z")
        after_chain(nc.vector.tensor_tensor(s_Z[:], ps_small[:, 1:2],
                                            s_pk[:], ALU.add))
        s_Zr = sb.tile([64, 1], f32, tag="Zr")
        after_chain(nc.vector.reciprocal(s_Zr[:], s_Z[:]))
        s_negZr = sb.tile([64, 1], f32, tag="negZr")
        after_chain(nc.vector.tensor_scalar_mul(s_negZr[:], s_Zr[:], -1.0))
        s_S1 = sb.tile([64, 1], f32, tag="S1")
        after_chain(nc.vector.tensor_copy(s_S1[:], ps_small[:, 2:3]))

        if stage <= 4:
            s_dbg = sb.tile([NM, D], f32, tag="dbg")
            nc.vector.tensor_scalar(s_dbg[:], s_p[0:NM, 0:D], s_Z[:],
                                    None, ALU.mult)
            nc.sync.dma_start(d_out[:], s_dbg[:])
            return

        # ---- combine: sigmoid(Zr*(S1*web + pk*fd + F)), exp-based ----
        s_t1 = sb.tile([NM, D], f32, tag="t1")
        nc.vector.tensor_scalar(s_t1[:], ps_fd[:], s_pk[:], None, ALU.mult)
        s_t2 = sb.tile([NM, D], f32, tag="t2")
        after_chain(nc.vector.tensor_scalar(s_t2[:], web, s_S1[:], None,
                                            ALU.mult))
        s_t3 = sb.tile([NM, D], f32, tag="t3")
        after_chain(nc.vector.tensor_add(s_t3[:], s_t1[:], s_t2[:]))
        s_t4 = sb.tile([NM, D], f32, tag="t4")
        after_chain(nc.vector.tensor_add(s_t4[:], s_t3[:], ps_F[:]))
        # sigmoid(x) = 1/(1+exp(-x)); x = t4 * Zr (avoids Sigmoid table load)
        s_emq = sb.tile([NM, D], f32, tag="emq")
        nc.scalar.activation(s_emq[:], s_t4[:], AF.Exp, scale=s_negZr[:])
        s_den = sb.tile([NM, D], f32, tag="den")
        after_chain(nc.vector.tensor_scalar_add(s_den[:], s_emq[:], 1.0))
        s_out = sb.tile([NM, D], f32, tag="out_sb")
        after_chain(nc.vector.reciprocal(s_out[:], s_den[:]))
        nc.sync.dma_start(d_out[:], s_out[:])

    with tile.TileContext(nc) as tc:
        with tc.tile_pool(name="sb", bufs=1) as sb, \
             tc.tile_pool(name="ps", bufs=1, space="PSUM") as ps:
            _emit(tc, sb, ps)

    nc.compile()
    return nc


def _get_nc():
    if "nc" not in _cache:
        _cache["nc"] = _build()
    return _cache["nc"]


def _prep_core_inputs(h_src, h_dst, edge_feat, adj, W_src, W_dst, W_edge,
                      attn_l, attn_r):
    f32, f64 = np.float32, np.float64
    wl = (W_src.astype(f64) @ attn_l.astype(f64)).astype(f32)
    wr = (W_dst.astype(f64) @ attn_r.astype(f64)).astype(f32)
    s_e = float(np.dot(W_edge.astype(f64), attn_l.astype(f64)))
    s_e_safe = s_e if abs(s_e) > 1e-20 else 1e-20

    par = np.zeros((128, P_COLS), BF16)
    par[:, P_WL:P_WL + NM] = np.tile(wl[:, None], (1, NM))
    par[:, P_WS:P_WS + D] = W_src
    par[0:KD, P_WR:P_WR + D] = np.tile(wr[:, None], (1, D))
    par[0:KD, P_WC:P_WC + 1] = wr[:, None]
    par[0:KD, P_WD:P_WD + D] = W_dst
    par[0:NM, P_WE:P_WE + D] = np.tile(
        (W_edge.astype(f64) / s_e_safe).astype(f32)[None, :], (NM, 1))

    # fold the adjacency mask into the edge term: -1e30 where adj=0 makes
    # exp() zero those slots; pad rows (o >= 1000) also get -1e30
    ef_s = ((edge_feat.astype(f64) * s_e) +
            (adj.astype(f64) - 1.0) * BIG).astype(f32)  # (B, NO, NM)

    in_maps = []
    for b in range(B):
        parb = par.copy()
        parb[0:KD, P_HD:P_HD + NM] = h_dst[b].T
        efp = np.full((NOP, NM), -BIG, f32)
        efp[0:NO] = ef_s[b]
        # (o=128c+p, m) -> ea[p, 64c+m]
        ea = np.ascontiguousarray(
            efp.reshape(NC, 128, NM).transpose(1, 0, 2).reshape(128, SW)
        ).astype(BF16)
        hsT = np.zeros((KS, NOP), BF16)
        hsT[:, 0:NO] = h_src[b].T
        in_maps.append({"hsT": hsT, "ea": ea, "par": parb})
    return in_maps


def kernel(**inputs):
    if "ope_ma_adj_batch" in inputs and "adj" not in inputs:
        inputs = dict(inputs)
        inputs["adj"] = inputs.pop("ope_ma_adj_batch")
    args = {k: np.asarray(inputs[k]) for k in
            ("h_src", "h_dst", "edge_feat", "adj", "W_src", "W_dst", "W_edge",
             "attn_l", "attn_r")}

    from concourse.bass_utils import run_bass_kernel_spmd

    nc = _get_nc()
    in_maps = _prep_core_inputs(**args)
    res = run_bass_kernel_spmd(nc, in_maps, core_ids=list(range(N_CORES)))
    out = np.stack([res.results[b]["out"] for b in range(B)], axis=0)
    return out.astype(np.float32)


# revision 13
# speedup vs baseline: 1.7728x; 1.1211x over previous
"""GATedge kernel for Trainium2, 8 NeuronCores, batch-parallel (1 batch elem / core).

v2: o-on-partitions layout - transpose-free.

Math (per batch b), restructured from the reference:
  s_e  = dot(W_edge, attn_l); wl = W_src @ attn_l; wr = W_dst @ attn_r  (host)
  score[o,m] = el[o] + er[m] + efm[o,m]   (adj multiply dropped: masked
    slots carry -1e30 in efm so their pre-mask value is irrelevant)
  p    = exp(max(score, 0.2*score))       (lrelu via max)
  pk   = exp(lrelu(2*er));  Z[m] = sum_o p + pk
  F^T  = sum_c fs_c^T @ p_c  (fs = h_src @ W_src)   [PE, o-contraction]
  S1   = g^T @ 1, g = p * efm;  Zcol = p^T @ 1      [PE ones-matmuls]
  out  = sigmoid((S1*W_edge/s_e + F + pk*feat_dst) / Z)

Layout: o padded to 1024 = 8 chunks x 128 partitions. Score tensors are
(128, 512): block c cols [64c,64c+64) hold o-chunk c (partition p = o-128c).
el enters the score PSUM via matmuls with lhsT=hsT_c, rhs=wl tiled 64 wide
(broadcast across m); er via lhsT=wr tiled 128 wide, rhs=hdT (broadcast
across partitions). Pad rows carry efm=-1e30 so p=0 there.
F^T (d-part, m-free) needs ONE transpose back to m-part for the combine.

All large tensors bf16 (rel err 1.3e-2 vs the 2e-2 gate); PSUM fp32.

Hardware rules learned the hard way (violations crash the exec unit):
 - never read PSUM rows a matmul didn't write
 - a PE transpose must be followed by its PSUM->SBUF copy before the next
   transpose into the same bank
 - tensor_tensor with both SBUF inputs needs equal base partitions
 - ACT Lrelu gives wrong results (alpha semantics) - use max(x, 0.2x)
"""

import sys

for _p in ("/opt/trn_rl_repo", "/root/.axon_site/_ro/trn_rl_repo"):
    if _p not in sys.path:
        sys.path.insert(0, _p)

import numpy as np
import ml_dtypes

BF16 = ml_dtypes.bfloat16

B, NO, NM, KS, KD, D = 8, 1000, 64, 128, 64, 128
NOP = 1024                # o padded
NC = 8                    # o-chunks
SW = NC * NM              # score width = 512
N_CORES = 8
NEG_SLOPE = 0.2
BIG = 1e30

# packed param buffer column layout
P_WL, P_WS, P_WR, P_HD, P_WC, P_WD, P_WE = 0, 64, 192, 320, 384, 385, 513
P_COLS = 641

_cache = {}


def _build():
    import os
    import concourse.tile as tile
    from concourse import bacc, mybir
    from concourse.tile import add_dep_helper

    stage = int(os.environ.get("BASS_GAT_STAGE", "9"))
    f32 = mybir.dt.float32
    bf = mybir.dt.bfloat16
    AF = mybir.ActivationFunctionType
    ALU = mybir.AluOpType

    nc = bacc.Bacc("TRN2", target_bir_lowering=False, debug=False,
                   num_devices=N_CORES)

    d_hsT = nc.dram_tensor("hsT", [KS, NOP], bf, kind="ExternalInput")
    d_ea = nc.dram_tensor("ea", [128, SW], bf, kind="ExternalInput")
    d_par = nc.dram_tensor("par", [128, P_COLS], bf, kind="ExternalInput")
    d_out = nc.dram_tensor("out", [NM, D], f32, kind="ExternalOutput")

    def _emit(tc, sb, ps):
        # ---- input DMAs: par0(SP), hsT0(Pool swdge), ea0(ACT), hsT1(SP),
        # ea1(Pool swdge), par1(SP) ----
        s_par = sb.tile([128, P_COLS], bf, tag="par")
        nc.sync.dma_start(s_par[:, 0:P_WD], d_par[:, 0:P_WD])
        s_hsT = sb.tile([KS, NOP], bf, tag="hsT")
        nc.gpsimd.dma_start(s_hsT[:, 0:512], d_hsT[:, 0:512])
        s_ea = sb.tile([128, SW], bf, tag="ea")
        nc.scalar.dma_start(s_ea[:, 0:256], d_ea[:, 0:256])
        nc.sync.dma_start(s_hsT[:, 512:NOP], d_hsT[:, 512:NOP])
        nc.gpsimd.dma_start(s_ea[:, 256:SW], d_ea[:, 256:SW])
        nc.sync.dma_start(s_par[:, P_WD:P_COLS], d_par[:, P_WD:P_COLS])

        wl64 = s_par[:, P_WL:P_WL + NM]
        wsrc = s_par[:, P_WS:P_WS + D]
        wr128 = s_par[0:KD, P_WR:P_WR + D]
        hdT = s_par[0:KD, P_HD:P_HD + NM]
        wrcol = s_par[0:KD, P_WC:P_WC + 1]
        wdst = s_par[0:KD, P_WD:P_WD + D]
        web = s_par[0:NM, P_WE:P_WE + D]

        s_ones = sb.tile([128, 1], bf, tag="ones")
        nc.gpsimd.memset(s_ones[:], 1.0)
        # 128x128 identity for the final F^T -> F transpose
        s_ident = sb.tile([128, 128], bf, tag="ident")
        nc.gpsimd.memset(s_ident[:], 0.0)
        nc.gpsimd.affine_select(
            out=s_ident[:], in_=s_ident[:],
            compare_op=ALU.not_equal,
            fill=1.0, base=0, pattern=[[-1, 128]], channel_multiplier=1)

        # ---- PE: er column for ekk; per chunk er+el accumulate pair
        # (er start=True must be immediately followed by its el start=False:
        # an intervening start=True on the same PSUM bank resets the group).
        # score/fs psum split per half: Tile deps are tile-granular, so one
        # big tile would gate chunk-0 consumers on chunk-7 producers.
        ps_small = ps.tile([KD, 3], f32, tag="small", padded_shape=[KD, 512])
        nc.tensor.matmul(ps_small[:, 0:1], hdT, wrcol, start=True, stop=True)

        ps_score = [ps.tile([128, SW // 2], f32, tag=f"score{i}",
                            name=f"ps_score{i}") for i in range(2)]
        ps_fs = [ps.tile([128, NC * D // 2], f32, tag=f"fs{i}",
                         name=f"ps_fs{i}") for i in range(2)]
        for c in range(NC):
            i, cc = c // 4, c % 4
            h_c = s_hsT[:, D * c:D * c + D]
            nc.tensor.matmul(ps_score[i][:, NM * cc:NM * cc + NM], wr128, hdT,
                             start=True, stop=False)
            nc.tensor.matmul(ps_score[i][:, NM * cc:NM * cc + NM], h_c, wl64,
                             start=False, stop=True)
            nc.tensor.matmul(ps_fs[i][:, D * cc:D * cc + D], h_c, wsrc,
                             start=True, stop=True)
        ps_fd = ps.tile([NM, D], f32, tag="fd", padded_shape=[NM, 512])
        nc.tensor.matmul(ps_fd[:], hdT, wdst, start=True, stop=True)

        if stage <= 1:
            s_dbg = sb.tile([NM, D], f32, tag="dbg")
            nc.vector.tensor_copy(s_dbg[:], ps_score[0][0:NM, 0:D])
            nc.sync.dma_start(d_out[:], s_dbg[:])
            return

        # ekk = lrelu(2*er) prologue - off critical path (needs par only);
        # emitted first so these tiny ops sit ahead of the score chain in
        # the DVE/ACT queues
        s_er2 = sb.tile([64, 1], f32, tag="er2")
        nc.scalar.mul(s_er2[:], ps_small[:, 0:1], 2.0)
        s_er2s = sb.tile([64, 1], f32, tag="er2s")
        nc.vector.tensor_scalar_mul(s_er2s[:], s_er2[:], NEG_SLOPE)
        s_ekk = sb.tile([64, 1], f32, tag="ekk")
        nc.vector.tensor_max(s_ekk[:], s_er2[:], s_er2s[:])
        s_pk = sb.tile([64, 1], f32, tag="pk")
        nc.scalar.activation(s_pk[:], s_ekk[:], AF.Exp)

        # ---- score chain, 2 column chunks of 256; fs copies interleave
        # on ACT (fs half i right before exp i keeps them off the DVE) ----
        s_fs = sb.tile([128, NC * D], bf, tag="fs_sb")
        s_a2 = sb.tile([128, SW], bf, tag="a2")
        s_ay = sb.tile([128, SW], bf, tag="ay")
        s_eijk = sb.tile([128, SW], bf, tag="eijk")
        s_p = sb.tile([128, SW], bf, tag="p")
        r_exp = [None, None]
        r_last = None
        for i in range(2):
            c = slice(256 * i, 256 * i + 256)
            nc.scalar.copy(s_fs[:, 512 * i:512 * i + 512], ps_fs[i][:])
            r_a = nc.vector.tensor_tensor(s_a2[:, c], ps_score[i][:],
                                          s_ea[:, c], ALU.add)
            if r_last is not None:
                add_dep_helper(r_a.ins, r_last.ins,
                               reason="score chain chunk order on DVE")
            nc.vector.tensor_scalar_mul(s_ay[:, c], s_a2[:, c], NEG_SLOPE)
            r_last = nc.vector.tensor_max(s_eijk[:, c], s_a2[:, c],
                                          s_ay[:, c])
            r_exp[i] = nc.scalar.activation(s_p[:, c], s_eijk[:, c], AF.Exp)

        def after_chain(r):
            add_dep_helper(r.ins, r_last.ins,
                           reason="defer small DVE op past score chain")
            return r

        if stage <= 3:
            s_dbg = sb.tile([NM, D], f32, tag="dbg")
            nc.vector.tensor_copy(s_dbg[:], s_p[0:NM, 0:D])
            nc.sync.dma_start(d_out[:], s_dbg[:])
            return

        # g = p * efm (for S1), per chunk on DVE right after each exp
        s_g = sb.tile([128, SW], bf, tag="g")
        for i in range(2):
            c = slice(256 * i, 256 * i + 256)
            after_chain(nc.vector.tensor_tensor(s_g[:, c], s_p[:, c],
                                                s_ea[:, c], ALU.mult))

        if stage == 31:
            s_dbg = sb.tile([NM, D], f32, tag="dbg")
            nc.vector.tensor_copy(s_dbg[:], s_g[0:NM, 0:D])
            nc.sync.dma_start(d_out[:], s_dbg[:])
            return

        # ---- PE: F^T, Z, S1 accumulating matmuls per chunk ----
        ps_FT = ps.tile([D, NM], f32, tag="FT", padded_shape=[D, 512])
        for c in range(NC):
            nc.tensor.matmul(ps_FT[:], s_fs[:, D * c:D * c + D],
                             s_p[:, NM * c:NM * c + NM],
                             start=(c == 0), stop=(c == NC - 1))
        if stage == 32:
            s_dbg = sb.tile([NM, D], f32, tag="dbg")
            nc.gpsimd.memset(s_dbg[:], 0.0)
            nc.vector.tensor_copy(s_dbg[:, 0:NM], ps_FT[0:NM, 0:NM])
            nc.sync.dma_start(d_out[:], s_dbg[:])
            return
        for c in range(NC):
            nc.tensor.matmul(ps_small[:, 1:2], s_p[:, NM * c:NM * c + NM],
                             s_ones[:], start=(c == 0), stop=(c == NC - 1))
        for c in range(NC):
            nc.tensor.matmul(ps_small[:, 2:3], s_g[:, NM * c:NM * c + NM],
                             s_ones[:], start=(c == 0), stop=(c == NC - 1))
        if stage == 33:
            s_dbg = sb.tile([NM, D], f32, tag="dbg")
            nc.gpsimd.memset(s_dbg[:], 0.0)
            nc.vector.tensor_copy(s_dbg[:, 0:3], ps_small[:, 0:3])
            nc.sync.dma_start(d_out[:], s_dbg[:])
            return

        # F^T -> F (one transpose via PE)
        s_FT = sb.tile([D, NM], bf, tag="FT_sb")
        nc.vector.tensor_copy(s_FT[:], ps_FT[:])
        ps_F = ps.tile([NM, D], bf, tag="F", padded_shape=[NM, 512])
        nc.tensor.transpose(ps_F[:], s_FT[:], s_ident[:])

        # Z = Zcol + pk; Zr = 1/Z; negZr; S1 copy
        s_Z = sb.tile([64, 1], f32, tag="Z")
        after_chain(nc.vector.tensor_tensor(s_Z[:], ps_small[:, 1:2],
                                            s_pk[:], ALU.add))
        s_Zr = sb.tile([64, 1], f32, tag="Zr")
        after_chain(nc.vector.reciprocal(s_Zr[:], s_Z[:]))
        s_negZr = sb.tile([64, 1], f32, tag="negZr")
        after_chain(nc.vector.tensor_scalar_mul(s_negZr[:], s_Zr[:], -1.0))
        s_S1 = sb.tile([64, 1], f32, tag="S1")
        after_chain(nc.vector.tensor_copy(s_S1[:], ps_small[:, 2:3]))

        if stage <= 4:
            s_dbg = sb.tile([NM, D], f32, tag="dbg")
            nc.vector.tensor_scalar(s_dbg[:], s_p[0:NM, 0:D], s_Z[:],
                                    None, ALU.mult)
            nc.sync.dma_start(d_out[:], s_dbg[:])
            return

        # ---- combine: sigmoid(Zr*(S1*web + pk*fd + F)), exp-based ----
        s_t1 = sb.tile([NM, D], f32, tag="t1")
        nc.vector.tensor_scalar(s_t1[:], ps_fd[:], s_pk[:], None, ALU.mult)
        s_t2 = sb.tile([NM, D], f32, tag="t2")
        after_chain(nc.vector.tensor_scalar(s_t2[:], web, s_S1[:], None,
                                            ALU.mult))
        s_t3 = sb.tile([NM, D], f32, tag="t3")
        after_chain(nc.vector.tensor_add(s_t3[:], s_t1[:], s_t2[:]))
        s_t4 = sb.tile([NM, D], f32, tag="t4")
        after_chain(nc.vector.tensor_add(s_t4[:], s_t3[:], ps_F[:]))
        # sigmoid(x) = 1/(1+exp(-x)); x = t4 * Zr (avoids Sigmoid table load)
        s_emq = sb.tile([NM, D], f32, tag="emq")
        nc.scalar.activation(s_emq[:], s_t4[:], AF.Exp, scale=s_negZr[:])
        s_den = sb.tile([NM, D], f32, tag="den")
        after_chain(nc.vector.tensor_scalar_add(s_den[:], s_emq[:], 1.0))
        s_out = sb.tile([NM, D], f32, tag="out_sb")
        after_chain(nc.vector.reciprocal(s_out[:], s_den[:]))
        nc.sync.dma_start(d_out[:], s_out[:])

    with tile.TileContext(nc) as tc:
        with tc.tile_pool(name="sb", bufs=1) as sb, \
             tc.tile_pool(name="ps", bufs=1, space="PSUM") as ps:
            _emit(tc, sb, ps)

    nc.compile()
    return nc


def _get_nc():
    if "nc" not in _cache:
        _cache["nc"] = _build()
    return _cache["nc"]


def _prep_core_inputs(h_src, h_dst, edge_feat, adj, W_src, W_dst, W_edge,
                      attn_l, attn_r):
    f32, f64 = np.float32, np.float64
    wl = (W_src.astype(f64) @ attn_l.astype(f64)).astype(f32)
    wr = (W_dst.astype(f64) @ attn_r.astype(f64)).astype(f32)
    s_e = float(np.dot(W_edge.astype(f64), attn_l.astype(f64)))
    s_e_safe = s_e if abs(s_e) > 1e-20 else 1e-20

    par = np.zeros((128, P_COLS), BF16)
    par[:, P_WL:P_WL + NM] = np.tile(wl[:, None], (1, NM))
    par[:, P_WS:P_WS + D] = W_src
    par[0:KD, P_WR:P_WR + D] = np.tile(wr[:, None], (1, D))
    par[0:KD, P_WC:P_WC + 1] = wr[:, None]
    par[0:KD, P_WD:P_WD + D] = W_dst
    par[0:NM, P_WE:P_WE + D] = np.tile(
        (W_edge.astype(f64) / s_e_safe).astype(f32)[None, :], (NM, 1))

    # fold the adjacency mask into the edge term: -1e30 where adj=0 makes
    # exp() zero those slots; pad rows (o >= 1000) also get -1e30
    ef_s = ((edge_feat.astype(f64) * s_e) +
            (adj.astype(f64) - 1.0) * BIG).astype(f32)  # (B, NO, NM)

    in_maps = []
    for b in range(B):
        parb = par.copy()
        parb[0:KD, P_HD:P_HD + NM] = h_dst[b].T
        efp = np.full((NOP, NM), -BIG, f32)
        efp[0:NO] = ef_s[b]
        # (o=128c+p, m) -> ea[p, 64c+m]
        ea = np.ascontiguousarray(
            efp.reshape(NC, 128, NM).transpose(1, 0, 2).reshape(128, SW)
        ).astype(BF16)
        hsT = np.zeros((KS, NOP), BF16)
        hsT[:, 0:NO] = h_src[b].T
        in_maps.append({"hsT": hsT, "ea": ea, "par": parb})
    return in_maps


def kernel(**inputs):
    if "ope_ma_adj_batch" in inputs and "adj" not in inputs:
        inputs = dict(inputs)
        inputs["adj"] = inputs.pop("ope_ma_adj_batch")
    args = {k: np.asarray(inputs[k]) for k in
            ("h_src", "h_dst", "edge_feat", "adj", "W_src", "W_dst", "W_edge",
             "attn_l", "attn_r")}

    from concourse.bass_utils import run_bass_kernel_spmd

    nc = _get_nc()
    in_maps = _prep_core_inputs(**args)
    res = run_bass_kernel_spmd(nc, in_maps, core_ids=list(range(N_CORES)))
    out = np.stack([res.results[b]["out"] for b in range(B)], axis=0)
    return out.astype(np.float32)


# revision 20
# speedup vs baseline: 1.7866x; 1.0078x over previous
"""GATedge kernel for Trainium2, 8 NeuronCores, batch-parallel (1 batch elem / core).

v2: o-on-partitions layout - transpose-free.

Math (per batch b), restructured from the reference:
  s_e  = dot(W_edge, attn_l); wl = W_src @ attn_l; wr = W_dst @ attn_r  (host)
  score[o,m] = el[o] + er[m] + efm[o,m]   (adj multiply dropped: masked
    slots carry -1e30 in efm so their pre-mask value is irrelevant)
  p    = exp(max(score, 0.2*score))       (lrelu via max)
  pk   = exp(lrelu(2*er));  Z[m] = sum_o p + pk
  F^T  = sum_c fs_c^T @ p_c  (fs = h_src @ W_src)   [PE, o-contraction]
  S1   = g^T @ 1, g = p * efm;  Zcol = p^T @ 1      [PE ones-matmuls]
  out  = sigmoid((S1*W_edge/s_e + F + pk*feat_dst) / Z)

Layout: o padded to 1024 = 8 chunks x 128 partitions. Score tensors are
(128, 512): block c cols [64c,64c+64) hold o-chunk c (partition p = o-128c).
el enters the score PSUM via matmuls with lhsT=hsT_c, rhs=wl tiled 64 wide
(broadcast across m); er via lhsT=wr tiled 128 wide, rhs=hdT (broadcast
across partitions). Pad rows carry efm=-1e30 so p=0 there.
F^T (d-part, m-free) needs ONE transpose back to m-part for the combine.

All large tensors bf16 (rel err 1.3e-2 vs the 2e-2 gate); PSUM fp32.

Hardware rules learned the hard way (violations crash the exec unit):
 - never read PSUM rows a matmul didn't write
 - a PE transpose must be followed by its PSUM->SBUF copy before the next
   transpose into the same bank
 - tensor_tensor with both SBUF inputs needs equal base partitions
 - ACT Lrelu gives wrong results (alpha semantics) - use max(x, 0.2x)
"""

import sys

for _p in ("/opt/trn_rl_repo", "/root/.axon_site/_ro/trn_rl_repo"):
    if _p not in sys.path:
        sys.path.insert(0, _p)

import numpy as np
import ml_dtypes

BF16 = ml_dtypes.bfloat16

B, NO, NM, KS, KD, D = 8, 1000, 64, 128, 64, 128
NOP = 1024                # o padded
NC = 8                    # o-chunks
SW = NC * NM              # score width = 512
N_CORES = 8
NEG_SLOPE = 0.2
BIG = 1e30

# packed param buffer column layout (par0 = cols [0, P_WD); par1 = rest)
P_WL, P_WS, P_WR, P_HD, P_WC, P_ON, P_ID = 0, 64, 192, 320, 384, 385, 386
P_WD, P_WE = 514, 642
P_COLS = 770

_cache = {}


def _build():
    import os
    import concourse.tile as tile
    from concourse import bacc, mybir
    from concourse.tile import add_dep_helper

    stage = int(os.environ.get("BASS_GAT_STAGE", "9"))
    f32 = mybir.dt.float32
    bf = mybir.dt.bfloat16
    AF = mybir.ActivationFunctionType
    ALU = mybir.AluOpType

    nc = bacc.Bacc("TRN2", target_bir_lowering=False, debug=False,
                   num_devices=N_CORES)

    d_hsT = nc.dram_tensor("hsT", [KS, NOP], bf, kind="ExternalInput")
    d_ea = nc.dram_tensor("ea", [128, SW], bf, kind="ExternalInput")
    d_par = nc.dram_tensor("par", [128, P_COLS], bf, kind="ExternalInput")
    d_out = nc.dram_tensor("out", [NM, D], f32, kind="ExternalOutput")

    def _emit(tc, sb, ps):
        # ---- input DMAs: par0(SP), hsT0(Pool swdge), ea0(ACT), hsT1(SP),
        # ea1(Pool swdge), par1(SP) ----
        s_par = sb.tile([128, P_COLS], bf, tag="par")
        nc.sync.dma_start(s_par[:, 0:P_WD], d_par[:, 0:P_WD])
        s_hsT = sb.tile([KS, NOP], bf, tag="hsT")
        nc.gpsimd.dma_start(s_hsT[:, 0:512], d_hsT[:, 0:512])
        s_ea = sb.tile([128, SW], bf, tag="ea")
        nc.scalar.dma_start(s_ea[:, 0:256], d_ea[:, 0:256])
        nc.sync.dma_start(s_hsT[:, 512:NOP], d_hsT[:, 512:NOP])
        nc.gpsimd.dma_start(s_ea[:, 256:SW], d_ea[:, 256:SW])
        nc.sync.dma_start(s_par[:, P_WD:P_COLS], d_par[:, P_WD:P_COLS])

        wl64 = s_par[:, P_WL:P_WL + NM]
        wsrc = s_par[:, P_WS:P_WS + D]
        wr128 = s_par[0:KD, P_WR:P_WR + D]
        hdT = s_par[0:KD, P_HD:P_HD + NM]
        wrcol = s_par[0:KD, P_WC:P_WC + 1]
        s_ones = s_par[:, P_ON:P_ON + 1]
        s_ident = s_par[:, P_ID:P_ID + D]   # host-provided 128x128 identity
        wdst = s_par[0:KD, P_WD:P_WD + D]
        web = s_par[0:NM, P_WE:P_WE + D]

        # ---- PE: er column for ekk; per chunk er+el accumulate pair
        # (er start=True must be immediately followed by its el start=False:
        # an intervening start=True on the same PSUM bank resets the group).
        # score/fs psum split per half: Tile deps are tile-granular, so one
        # big tile would gate chunk-0 consumers on chunk-7 producers.
        ps_small = ps.tile([KD, 3], f32, tag="small", padded_shape=[KD, 512])
        nc.tensor.matmul(ps_small[:, 0:1], hdT, wrcol, start=True, stop=True)

        # per block: [er, el, ea] accumulate triple (must stay adjacent: a
        # later start=True in the same PSUM bank resets open groups), ea via
        # identity-matmul so the full score lands in PSUM with no DVE add.
        ps_score = [ps.tile([128, SW // 2], f32, tag=f"score{i}",
                            name=f"ps_score{i}") for i in range(2)]
        ps_fs = [ps.tile([128, NC * D // 2], f32, tag=f"fs{i}",
                         name=f"ps_fs{i}") for i in range(2)]
        for i in range(2):
            for cc in range(4):
                c = 4 * i + cc
                h_c = s_hsT[:, D * c:D * c + D]
                blk = ps_score[i][:, NM * cc:NM * cc + NM]
                nc.tensor.matmul(blk, wr128, hdT, start=True, stop=False)
                nc.tensor.matmul(blk, h_c, wl64, start=False, stop=False)
                nc.tensor.matmul(blk, s_ident, s_ea[:, NM * c:NM * c + NM],
                                 start=False, stop=True)
            for cc in range(4):
                c = 4 * i + cc
                h_c = s_hsT[:, D * c:D * c + D]
                nc.tensor.matmul(ps_fs[i][:, D * cc:D * cc + D], h_c, wsrc,
                                 start=True, stop=True)
        ps_fd = ps.tile([NM, D], f32, tag="fd", padded_shape=[NM, 512])
        nc.tensor.matmul(ps_fd[:], hdT, wdst, start=True, stop=True)

        if stage <= 1:
            s_dbg = sb.tile([NM, D], f32, tag="dbg")
            nc.vector.tensor_copy(s_dbg[:], ps_score[0][0:NM, 0:D])
            nc.sync.dma_start(d_out[:], s_dbg[:])
            return

        # ekk = lrelu(2*er) prologue - off critical path (needs par only);
        # emitted first so these tiny ops sit ahead of the score chain in
        # the DVE/ACT queues
        s_er2 = sb.tile([64, 1], f32, tag="er2")
        nc.scalar.mul(s_er2[:], ps_small[:, 0:1], 2.0)
        s_er2s = sb.tile([64, 1], f32, tag="er2s")
        nc.vector.tensor_scalar_mul(s_er2s[:], s_er2[:], NEG_SLOPE)
        s_ekk = sb.tile([64, 1], f32, tag="ekk")
        nc.vector.tensor_max(s_ekk[:], s_er2[:], s_er2s[:])
        s_pk = sb.tile([64, 1], f32, tag="pk")
        nc.scalar.activation(s_pk[:], s_ekk[:], AF.Exp)

        # ---- score chain, 2 column chunks of 256: lrelu then exp.
        # prelu=1: single ACT Prelu from PSUM; else DVE mul+max fallback.
        prelu = os.environ.get("BASS_GAT_PRELU", "1") == "1"
        s_fs = sb.tile([128, NC * D], bf, tag="fs_sb")
        s_ay = sb.tile([128, SW], bf, tag="ay")
        s_eijk = sb.tile([128, SW], bf, tag="eijk")
        s_p = sb.tile([128, SW], bf, tag="p")
        r_exp = [None, None]
        r_last = None
        for i in range(2):
            c = slice(256 * i, 256 * i + 256)
            if prelu:
                r_last = nc.scalar.activation(s_eijk[:, c], ps_score[i][:],
                                              AF.Prelu, alpha=NEG_SLOPE)
            else:
                r_a = nc.vector.tensor_scalar_mul(s_ay[:, c], ps_score[i][:],
                                                  NEG_SLOPE)
                if r_last is not None:
                    add_dep_helper(r_a.ins, r_last.ins,
                                   reason="score chain chunk order on DVE")
                r_last = nc.vector.tensor_max(s_eijk[:, c], ps_score[i][:],
                                              s_ay[:, c])
            r_exp[i] = nc.scalar.activation(s_p[:, c], s_eijk[:, c], AF.Exp)
            nc.scalar.copy(s_fs[:, 512 * i:512 * i + 512], ps_fs[i][:])

        def after_chain(r):
            add_dep_helper(r.ins, r_last.ins,
                           reason="defer small DVE op past score chain")
            return r

        if stage <= 3:
            s_dbg = sb.tile([NM, D], f32, tag="dbg")
            nc.vector.tensor_copy(s_dbg[:], s_p[0:NM, 0:D])
            nc.sync.dma_start(d_out[:], s_dbg[:])
            return

        # g = p * efm (for S1), per chunk on DVE right after each exp
        s_g = sb.tile([128, SW], bf, tag="g")
        for i in range(2):
            c = slice(256 * i, 256 * i + 256)
            after_chain(nc.vector.tensor_tensor(s_g[:, c], s_p[:, c],
                                                s_ea[:, c], ALU.mult))

        if stage == 31:
            s_dbg = sb.tile([NM, D], f32, tag="dbg")
            nc.vector.tensor_copy(s_dbg[:], s_g[0:NM, 0:D])
            nc.sync.dma_start(d_out[:], s_dbg[:])
            return

        # ---- PE: F^T, Z, S1 accumulating matmuls per chunk ----
        ps_FT = ps.tile([D, NM], f32, tag="FT", padded_shape=[D, 512])
        for c in range(NC):
            nc.tensor.matmul(ps_FT[:], s_fs[:, D * c:D * c + D],
                             s_p[:, NM * c:NM * c + NM],
                             start=(c == 0), stop=(c == NC - 1))
        if stage == 32:
            s_dbg = sb.tile([NM, D], f32, tag="dbg")
            nc.gpsimd.memset(s_dbg[:], 0.0)
            nc.vector.tensor_copy(s_dbg[:, 0:NM], ps_FT[0:NM, 0:NM])
            nc.sync.dma_start(d_out[:], s_dbg[:])
            return
        for c in range(NC):
            nc.tensor.matmul(ps_small[:, 1:2], s_p[:, NM * c:NM * c + NM],
                             s_ones[:], start=(c == 0), stop=(c == NC - 1))
        for c in range(NC):
            nc.tensor.matmul(ps_small[:, 2:3], s_g[:, NM * c:NM * c + NM],
                             s_ones[:], start=(c == 0), stop=(c == NC - 1))
        if stage == 33:
            s_dbg = sb.tile([NM, D], f32, tag="dbg")
            nc.gpsimd.memset(s_dbg[:], 0.0)
            nc.vector.tensor_copy(s_dbg[:, 0:3], ps_small[:, 0:3])
            nc.sync.dma_start(d_out[:], s_dbg[:])
            return

        # F^T -> F (one transpose via PE)
        s_FT = sb.tile([D, NM], bf, tag="FT_sb")
        nc.vector.tensor_copy(s_FT[:], ps_FT[:])
        ps_F = ps.tile([NM, D], bf, tag="F", padded_shape=[NM, 512])
        nc.tensor.transpose(ps_F[:], s_FT[:], s_ident[:])

        # Z = Zcol + pk; Zr = 1/Z; negZr; S1 copy
        s_Z = sb.tile([64, 1], f32, tag="Z")
        after_chain(nc.vector.tensor_tensor(s_Z[:], ps_small[:, 1:2],
                                            s_pk[:], ALU.add))
        s_Zr = sb.tile([64, 1], f32, tag="Zr")
        after_chain(nc.vector.reciprocal(s_Zr[:], s_Z[:]))
        s_hZr = sb.tile([64, 1], f32, tag="hZr")
        after_chain(nc.vector.tensor_scalar_mul(s_hZr[:], s_Zr[:], 0.5))
        s_S1 = sb.tile([64, 1], f32, tag="S1")
        after_chain(nc.vector.tensor_copy(s_S1[:], ps_small[:, 2:3]))

        if stage <= 4:
            s_dbg = sb.tile([NM, D], f32, tag="dbg")
            nc.vector.tensor_scalar(s_dbg[:], s_p[0:NM, 0:D], s_Z[:],
                                    None, ALU.mult)
            nc.sync.dma_start(d_out[:], s_dbg[:])
            return

        # ---- combine: sigmoid(Zr*(S1*web + pk*fd + F)) via tanh:
        # sigmoid(x) = 0.5 + 0.5*tanh(x/2)  (Tanh shares the Exp table) ----
        s_t1 = sb.tile([NM, D], f32, tag="t1")
        after_chain(nc.vector.tensor_scalar(s_t1[:], ps_fd[:], s_pk[:],
                                            None, ALU.mult))
        s_t2 = sb.tile([NM, D], f32, tag="t2")
        nc.gpsimd.tensor_scalar(s_t2[:], web, s_S1[:], None, ALU.mult)
        s_t3 = sb.tile([NM, D], f32, tag="t3")
        after_chain(nc.vector.tensor_tensor(s_t3[:], s_t1[:], ps_F[:],
                                            ALU.add))
        s_t4 = sb.tile([NM, D], f32, tag="t4")
        after_chain(nc.vector.tensor_add(s_t4[:], s_t3[:], s_t2[:]))
        s_th = sb.tile([NM, D], f32, tag="th")
        nc.scalar.activation(s_th[:], s_t4[:], AF.Tanh, scale=s_hZr[:])
        s_out = sb.tile([NM, D], f32, tag="out_sb")
        after_chain(nc.vector.tensor_scalar(s_out[:], s_th[:], 0.5, 0.5,
                                            ALU.mult, ALU.add))
        nc.sync.dma_start(d_out[:], s_out[:])

    with tile.TileContext(nc) as tc:
        with tc.tile_pool(name="sb", bufs=1) as sb, \
             tc.tile_pool(name="ps", bufs=1, space="PSUM") as ps:
            _emit(tc, sb, ps)

    nc.compile()
    return nc


def _get_nc():
    if "nc" not in _cache:
        _cache["nc"] = _build()
    return _cache["nc"]


def _prep_core_inputs(h_src, h_dst, edge_feat, adj, W_src, W_dst, W_edge,
                      attn_l, attn_r):
    f32, f64 = np.float32, np.float64
    wl = (W_src.astype(f64) @ attn_l.astype(f64)).astype(f32)
    wr = (W_dst.astype(f64) @ attn_r.astype(f64)).astype(f32)
    s_e = float(np.dot(W_edge.astype(f64), attn_l.astype(f64)))
    s_e_safe = s_e if abs(s_e) > 1e-20 else 1e-20

    par = np.zeros((128, P_COLS), BF16)
    par[:, P_WL:P_WL + NM] = np.tile(wl[:, None], (1, NM))
    par[:, P_WS:P_WS + D] = W_src
    par[0:KD, P_WR:P_WR + D] = np.tile(wr[:, None], (1, D))
    par[0:KD, P_WC:P_WC + 1] = wr[:, None]
    par[:, P_ON:P_ON + 1] = 1.0
    par[:, P_ID:P_ID + D] = np.eye(128, dtype=f32)
    par[0:KD, P_WD:P_WD + D] = W_dst
    par[0:NM, P_WE:P_WE + D] = np.tile(
        (W_edge.astype(f64) / s_e_safe).astype(f32)[None, :], (NM, 1))

    # fold the adjacency mask into the edge term: -1e30 where adj=0 makes
    # exp() zero those slots; pad rows (o >= 1000) also get -1e30
    ef_s = ((edge_feat.astype(f64) * s_e) +
            (adj.astype(f64) - 1.0) * BIG).astype(f32)  # (B, NO, NM)

    in_maps = []
    for b in range(B):
        parb = par.copy()
        parb[0:KD, P_HD:P_HD + NM] = h_dst[b].T
        efp = np.full((NOP, NM), -BIG, f32)
        efp[0:NO] = ef_s[b]
        # (o=128c+p, m) -> ea[p, 64c+m]
        ea = np.ascontiguousarray(
            efp.reshape(NC, 128, NM).transpose(1, 0, 2).reshape(128, SW)
        ).astype(BF16)
        hsT = np.zeros((KS, NOP), BF16)
        hsT[:, 0:NO] = h_src[b].T
        in_maps.append({"hsT": hsT, "ea": ea, "par": parb})
    return in_maps


def kernel(**inputs):
    if "ope_ma_adj_batch" in inputs and "adj" not in inputs:
        inputs = dict(inputs)
        inputs["adj"] = inputs.pop("ope_ma_adj_batch")
    args = {k: np.asarray(inputs[k]) for k in
            ("h_src", "h_dst", "edge_feat", "adj", "W_src", "W_dst", "W_edge",
             "attn_l", "attn_r")}

    from concourse.bass_utils import run_bass_kernel_spmd

    nc = _get_nc()
    in_maps = _prep_core_inputs(**args)
    res = run_bass_kernel_spmd(nc, in_maps, core_ids=list(range(N_CORES)))
    out = np.stack([res.results[b]["out"] for b in range(B)], axis=0)
    return out.astype(np.float32)
